# revision 15
# baseline (speedup 1.0000x reference)
"""CTC beam search decoder (beam_width=8, top_paths=1) on 8 Trainium2 cores.

Data-parallel: batch 1024 is split 128 rows per core; each core runs an
identical Bass kernel with the 128 batch rows mapped to the 128 SBUF
partitions. Per time step (T=128, sequential) each partition:
  - builds the CTC candidate scores: 8 merged-stay slots (slot w) plus
    8x62 extend slots laid out at 8 + i*64 + c (power-of-2 stride so the
    selection index decodes with shift/and),
  - tracks prefix identity with int32 node ids (pnode_k == node_i gives the
    CTC stay/extend merge; the merged extend slot is suppressed in-place via
    DVE match_replace on its exact fp32 value),
  - selects the top-8 with DVE max/max_index and records the raw selected
    slot indices as backpointers.
The host reconstructs the top-1 label sequence from the backpointers.

Device outputs per core: bp [128, T*8] int32 (selected slot per new beam),
fin [128, 8] fp32 (final log-prob ptot per beam). Host returns
(decoded [B,1,T] i32, lengths [B,1] i32, probability [B,1] f32) matching
the reference tuple.
"""

import sys

for _p in ("/opt/trn_rl_repo", "/root/.axon_site/_ro/trn_rl_repo"):
    if _p not in sys.path:
        sys.path.append(_p)

import numpy as np

B = 1024
T = 128
C = 63
V = C - 1
W = 8
NCORES = 8
BC = B // NCORES
NEG = -1e30
BIG = 1e30


def _build_nc():
    import concourse.bacc as bacc
    import concourse.mybir as mybir
    from concourse.tile import TileContext

    f32, i32, u32 = mybir.dt.float32, mybir.dt.int32, mybir.dt.uint32
    AO, AF, AX = mybir.AluOpType, mybir.ActivationFunctionType, mybir.AxisListType

    nc = bacc.Bacc("TRN2", debug=False, enable_asserts=False)
    lg_d = nc.dram_tensor("logits", [128, T * C], f32, kind="ExternalInput")
    bp_d = nc.dram_tensor("bp", [128, T * W], i32, kind="ExternalOutput")
    fin_d = nc.dram_tensor("fin", [128, W], f32, kind="ExternalOutput")

    with TileContext(nc) as tc:
        with tc.tile_pool(name="pp", bufs=1) as pp, tc.tile_pool(name="sp", bufs=2) as sp:
            D = nc.vector
            A = nc.scalar
            G = nc.gpsimd

            LG = pp.tile([128, T * C], f32)
            LOGP = pp.tile([128, T * C], f32)
            CAND = pp.tile([128, 528], f32)
            BP = pp.tile([128, T * W], i32)
            # f32 state pack: PNB|PB|PTOT|LASTF|ONES|BM|SPB|Z
            PACKF = pp.tile([128, 64], f32)
            # i32 state pack: LAST|NODE|PNODE
            S3 = pp.tile([128, 24], i32)
            IOTA62 = pp.tile([128, 62], i32)
            IOTA62F = pp.tile([128, 62], f32)
            IOTA8 = pp.tile([128, 8], i32)
            EXTV = pp.tile([128, 24], f32)   # [MAXV | NEG | MAXVdup]

            PNB = PACKF[:, 0:8]
            PB = PACKF[:, 8:16]
            PTOT = PACKF[:, 16:24]
            LASTF = PACKF[:, 24:32]
            ONES = PACKF[:, 32:40]
            BM = PACKF[:, 40:48]
            SPB = PACKF[:, 48:56]
            ZS = PACKF[:, 56:64]
            LAST = S3[:, 0:8]
            NODE = S3[:, 8:16]
            PNODE = S3[:, 16:24]

            G.iota(IOTA62[:], pattern=[[1, V]], base=0, channel_multiplier=0)
            G.iota(IOTA8[:], pattern=[[1, 8]], base=0, channel_multiplier=0)
            D.tensor_copy(IOTA62F[:], IOTA62[:])
            D.memset(EXTV[:, 8:16], NEG)
            D.memset(CAND[:], NEG)
            D.memset(ONES, 1.0)

            # log softmax over C, in chunks of TC_ timesteps
            TC_ = 16
            for ch in range(T // TC_):
                a, b = ch * TC_ * C, (ch + 1) * TC_ * C
                lgc = LG[:, a:b]
                nc.sync.dma_start(lgc, lg_d.ap()[:, a:b])
                MXc = pp.tile([128, TC_], f32, name=f"MXc{ch}")
                SMc = pp.tile([128, TC_], f32, name=f"SMc{ch}")
                EXc = pp.tile([128, TC_ * C], f32, name=f"EXc{ch}")
                lg3 = lgc.rearrange("p (t c) -> p t c", c=C)
                lp3 = LOGP[:, a:b].rearrange("p (t c) -> p t c", c=C)
                ex3 = EXc[:].rearrange("p (t c) -> p t c", c=C)
                D.tensor_reduce(MXc[:], lg3, AX.X, AO.max)
                D.tensor_tensor(lp3, lg3, MXc[:].unsqueeze(2).broadcast_to([128, TC_, C]), AO.subtract)
                A.activation(EXc[:], LOGP[:, a:b], AF.Exp)
                D.tensor_reduce(SMc[:], ex3, AX.X, AO.add)
                A.activation(SMc[:], SMc[:], AF.Ln)
                D.tensor_tensor(lp3, lp3, SMc[:].unsqueeze(2).broadcast_to([128, TC_, C]), AO.subtract)

            # state init (ptot kept as state; logaddexp(0,NEG)=0, else NEG)
            D.memset(PB, NEG)
            D.memset(PACKF[:, 8:9], 0.0)
            D.memset(PNB, NEG)
            D.memset(PTOT, NEG)
            D.memset(PACKF[:, 16:17], 0.0)
            D.memset(LAST, -1)
            D.tensor_copy(NODE, IOTA8[:])
            D.memset(PNODE, -1)
            D.memset(LASTF, -1.0)

            def lse(out_ap, a_ap, b_ap, tag):
                """out = max + ln(1 + exp(min - max)) (== logaddexp(a, b))."""
                mxv = sp.tile([128, 8], f32, name=f"mx_{tag}")
                mnv = sp.tile([128, 8], f32, name=f"mn_{tag}")
                D.tensor_tensor(mxv[:], a_ap, b_ap, AO.max)
                D.tensor_tensor(mnv[:], a_ap, b_ap, AO.min)
                G.tensor_tensor(mnv[:], mnv[:], mxv[:], AO.subtract)
                A.activation(mnv[:], mnv[:], AF.Exp)
                A.activation(mnv[:], mnv[:], AF.Ln, bias=1.0)
                G.tensor_tensor(out_ap, mxv[:], mnv[:], AO.add)

            # strided [w, c] views over the 8*64 extend region (pads untouched)
            def wcv(tile_ap):
                return tile_ap.rearrange("p (w c) -> p w c", c=64)[:, :, 0:V]

            ce3 = wcv(CAND[:, 8:520])
            CEFLAT = CAND[:, 8:520]

            for t in range(T):
                lp62 = LOGP[:, t * C: t * C + V]
                lpB = LOGP[:, t * C + V: t * C + C]
                lp62b = lp62.unsqueeze(1).broadcast_to([128, W, V])

                # stay-blank score piece (ptot is maintained state)
                D.tensor_tensor(SPB, PTOT, lpB.broadcast_to([128, 8]), AO.add)

                # last-char one-hot (f32 compare on gpsimd) / lp gather
                ISEQF = sp.tile([128, 512], f32)
                iseqf3 = wcv(ISEQF[:])
                D.tensor_tensor(iseqf3, LAST.unsqueeze(2).broadcast_to([128, W, V]),
                                IOTA62[:].unsqueeze(1).broadcast_to([128, W, V]), AO.is_equal)
                T496 = sp.tile([128, 512], f32)
                G.tensor_tensor(wcv(T496[:]), iseqf3, lp62b, AO.mult)
                LPPICK = sp.tile([128, 8], f32)
                D.tensor_reduce(LPPICK[:], wcv(T496[:]), AX.X, AO.add)
                SPNB = sp.tile([128, 8], f32)
                G.tensor_tensor(SPNB[:], PNB, LPPICK[:], AO.add)

                # extend plane: (is_rep ? pb : ptot) + lp
                PBB = sp.tile([128, 512], f32)
                G.tensor_tensor(wcv(PBB[:]), PB.unsqueeze(2).broadcast_to([128, W, V]),
                                lp62b, AO.add)
                G.tensor_tensor(ce3, PTOT.unsqueeze(2).broadcast_to([128, W, V]),
                                lp62b, AO.add)
                D.copy_predicated(ce3, iseqf3.bitcast(i32), wcv(PBB[:]))

                # parent match (pnode_k == node_i); gather [PB|PNB|PTOT|LASTF|ONES]
                M64 = sp.tile([128, 64], f32)
                m3 = M64[:].rearrange("p (k i) -> p k i", k=W)
                D.tensor_tensor(m3, PNODE.unsqueeze(2).broadcast_to([128, W, W]),
                                NODE.unsqueeze(1).broadcast_to([128, W, W]), AO.is_equal)
                T384 = sp.tile([128, 256], f32)
                t384v = T384[:].rearrange("p (k q i) -> p k q i", k=W, q=4)
                G.tensor_tensor(
                    t384v,
                    m3.unsqueeze(2).broadcast_to([128, W, 4, W]),
                    PACKF[:, 8:40].rearrange("p (q j) -> p q j", q=4).unsqueeze(1).broadcast_to([128, W, 4, W]),
                    AO.mult)
                MG = sp.tile([128, 32], f32)
                mgq = MG[:].rearrange("p (q k) -> p k q", q=4)
                D.tensor_reduce(mgq, t384v, AX.X, AO.add)
                PBP = MG[:, 0:8]
                PTP = MG[:, 8:16]
                LASTPF = MG[:, 16:24]
                MTD = MG[:, 24:32]
                REP = sp.tile([128, 8], i32)
                D.tensor_tensor(REP[:], LASTF, LASTPF, AO.is_equal)
                EB = sp.tile([128, 8], f32)
                D.tensor_copy(EB[:], PTP)
                D.copy_predicated(EB[:], REP[:], PBP)
                EADD = sp.tile([128, 8], f32)
                G.tensor_tensor(EADD[:], EB[:], LPPICK[:], AO.add)
                DLT = sp.tile([128, 8], f32)
                G.tensor_scalar(DLT[:], MTD, 1.0, BIG, AO.subtract, AO.mult)
                EADDM = sp.tile([128, 8], f32)
                G.tensor_tensor(EADDM[:], EADD[:], DLT[:], AO.add)

                # merged stay pnb and stay score (stay slots live at 8+k*64+62)
                lse(BM, SPNB[:], EADDM[:], "e")
                lse(ZS, SPB, BM, "f")
                D.tensor_copy(CAND[:, 8:520].rearrange("p (w c) -> p w c", c=64)[:, :, 62:63].squeeze(2), ZS)

                # suppression + top8 (backpointers written by max_index)
                D.match_replace(CEFLAT, EADDM[:], CEFLAT, NEG)
                MAXV = EXTV[:, 0:8]
                D.max(MAXV, CEFLAT)
                BPS = BP[:, t * W:(t + 1) * W]
                D.max_index(BPS.bitcast(u32), MAXV, CEFLAT)
                SEL = BPS.bitcast(i32)

                # decode selection: par = sel>>6 uniformly; ch-lane 62 marks stay
                PAR = sp.tile([128, 8], i32)
                D.tensor_scalar(PAR[:], SEL, 6, None, AO.arith_shift_right)
                EXTI = sp.tile([128, 24], i32)   # [CH | NEWID | NODEP]
                D.tensor_scalar(EXTI[:, 0:8], SEL, 63, None, AO.bitwise_and)
                ISSTAY = sp.tile([128, 8], i32)
                D.tensor_scalar(ISSTAY[:], EXTI[:, 0:8], V, None, AO.is_equal)

                # parent one-hot gathers (f32: [SPB|BM|Z], i32: [LAST|NODE|PNODE])
                PSEL = sp.tile([128, 64], i32)
                ps3 = PSEL[:].rearrange("p (w j) -> p w j", w=W)
                D.tensor_tensor(ps3, PAR[:].unsqueeze(2).broadcast_to([128, W, W]),
                                IOTA8[:].unsqueeze(1).broadcast_to([128, W, W]), AO.is_equal)
                PSELF = sp.tile([128, 64], f32)
                psf3 = PSELF[:].rearrange("p (w j) -> p w j", w=W)
                D.tensor_tensor(psf3, PAR[:].unsqueeze(2).broadcast_to([128, W, W]),
                                IOTA8[:].unsqueeze(1).broadcast_to([128, W, W]), AO.is_equal)
                TF192 = sp.tile([128, 192], f32)
                tf3 = TF192[:].rearrange("p (w q j) -> p w q j", w=W, q=3)
                G.tensor_tensor(
                    tf3,
                    PSELF[:].rearrange("p (w j) -> p w j", w=W).unsqueeze(2).broadcast_to([128, W, 3, W]),
                    PACKF[:, 40:64].rearrange("p (q j) -> p q j", q=3).unsqueeze(1).broadcast_to([128, W, 3, W]),
                    AO.mult)
                FG = sp.tile([128, 24], f32)
                fgq = FG[:].rearrange("p (q w) -> p w q", q=3)
                D.tensor_reduce(fgq, tf3, AX.X, AO.add)
                TI192 = sp.tile([128, 192], i32)
                ti3 = TI192[:].rearrange("p (w q j) -> p w q j", w=W, q=3)
                D.tensor_tensor(
                    ti3,
                    ps3.unsqueeze(2).broadcast_to([128, W, 3, W]),
                    S3[:].rearrange("p (q j) -> p q j", q=3).unsqueeze(1).broadcast_to([128, W, 3, W]),
                    AO.mult)
                IG = sp.tile([128, 24], i32)
                igq = IG[:].rearrange("p (q w) -> p w q", q=3)
                with nc.allow_low_precision(reason="int32 add reduce is exact"):
                    D.tensor_reduce(igq, ti3, AX.X, AO.add)

                # state update (packed selects; ext lanes of stays are dont-care)
                ISS24 = sp.tile([128, 24], i32)
                D.tensor_copy(ISS24[:].rearrange("p (q w) -> p q w", q=3),
                              ISSTAY[:].unsqueeze(1).broadcast_to([128, 3, 8]))
                D.tensor_scalar(EXTI[:, 8:16], SEL, (t + 1) * 1024, None, AO.add)
                D.tensor_copy(EXTI[:, 16:24], IG[:, 8:16])
                D.tensor_copy(EXTV[:, 16:24], MAXV)
                D.tensor_copy(S3[:], EXTI[:])
                D.copy_predicated(S3[:], ISS24[:], IG[:])
                D.tensor_copy(PACKF[:, 0:24], EXTV[:])
                D.copy_predicated(PACKF[:, 0:24], ISS24[:], FG[:])
                D.tensor_copy(LASTF, LAST)

            nc.sync.dma_start(fin_d.ap(), PTOT)
            nc.sync.dma_start(bp_d.ap(), BP[:])

    # The act-table chooser greedily picks the first table containing each
    # function, so alternating Exp/Ln thrashes between two tables (one
    # ~1.3us reload per activation). Restrict Exp and Ln to the combined
    # natural_log_exp_and_others set (real index preserved) so one resident
    # table serves both and the load hoists out of the loop.
    import concourse.bacc as bacc_mod
    _orig_gat = bacc_mod.get_activation_tables

    def _pinned_tables(arch):
        tabs = dict(_orig_gat(arch))
        both = mybir.ActivationFunctionType.Exp, mybir.ActivationFunctionType.Ln
        for name, s in tabs.items():
            if name != "natural_log_exp_and_others":
                tabs[name] = s - set(both)
        return tabs

    bacc_mod.get_activation_tables = _pinned_tables
    try:
        nc.compile()
    finally:
        bacc_mod.get_activation_tables = _orig_gat
    return nc


_NC_CACHE = None


def _get_nc():
    global _NC_CACHE
    if _NC_CACHE is None:
        _NC_CACHE = _build_nc()
    return _NC_CACHE


def _postprocess(bp, fin):
    """bp [rows, T*8] i32 selected slot ids; fin [rows, 8] f32 final ptot."""
    nb = bp.shape[0]
    sel = bp.reshape(nb, T, W)
    par = (sel >> 6).astype(np.int32)
    cl = sel & 63
    ch = np.where(cl == 62, -1, cl).astype(np.int32)
    dec = np.full((nb, T), -1, np.int32)
    lens = np.zeros((nb,), np.int32)
    w = np.zeros((nb,), np.int32)
    ar = np.arange(nb)
    chars = np.empty((nb, T), np.int32)
    for t in range(T - 1, -1, -1):
        chars[:, t] = ch[ar, t, w]
        w = par[ar, t, w]
    for b in range(nb):
        cs = chars[b][chars[b] >= 0]
        dec[b, :len(cs)] = cs
        lens[b] = len(cs)
    prob = np.exp(fin[:, 0].astype(np.float32))
    return dec, lens, prob


def kernel(logits):
    from concourse.bass_utils import run_bass_kernel_spmd

    logits = np.ascontiguousarray(np.asarray(logits, dtype=np.float32))
    assert logits.shape == (B, T, C), logits.shape
    nc = _get_nc()
    in_maps = [
        {"logits": np.ascontiguousarray(logits[c * BC:(c + 1) * BC].reshape(BC, T * C))}
        for c in range(NCORES)
    ]
    res = run_bass_kernel_spmd(nc, in_maps, core_ids=list(range(NCORES)))
    decoded = np.full((B, 1, T), -1, np.int32)
    lengths = np.zeros((B, 1), np.int32)
    probability = np.zeros((B, 1), np.float32)
    for c in range(NCORES):
        out = res.results[c]
        dec, lens, prob = _postprocess(out["bp"], out["fin"])
        decoded[c * BC:(c + 1) * BC, 0] = dec
        lengths[c * BC:(c + 1) * BC, 0] = lens
        probability[c * BC:(c + 1) * BC, 0] = prob
    return decoded, lengths, probability


# revision 16
# speedup vs baseline: 1.0870x; 1.0870x over previous
"""CTC beam search decoder (beam_width=8, top_paths=1) on 8 Trainium2 cores.

Data-parallel: batch 1024 is split 128 rows per core; each core runs an
identical Bass kernel with the 128 batch rows mapped to the 128 SBUF
partitions. Per time step (T=128, sequential) each partition:
  - builds the CTC candidate scores: 8 merged-stay slots (slot w) plus
    8x62 extend slots laid out at 8 + i*64 + c (power-of-2 stride so the
    selection index decodes with shift/and),
  - tracks prefix identity with int32 node ids (pnode_k == node_i gives the
    CTC stay/extend merge; the merged extend slot is suppressed in-place via
    DVE match_replace on its exact fp32 value),
  - selects the top-8 with DVE max/max_index and records the raw selected
    slot indices as backpointers.
The host reconstructs the top-1 label sequence from the backpointers.

Device outputs per core: bp [128, T*8] int32 (selected slot per new beam),
fin [128, 8] fp32 (final log-prob ptot per beam). Host returns
(decoded [B,1,T] i32, lengths [B,1] i32, probability [B,1] f32) matching
the reference tuple.
"""

import sys

for _p in ("/opt/trn_rl_repo", "/root/.axon_site/_ro/trn_rl_repo"):
    if _p not in sys.path:
        sys.path.append(_p)

import numpy as np

B = 1024
T = 128
C = 63
V = C - 1
W = 8
NCORES = 8
BC = B // NCORES
NEG = -1e30
BIG = 1e30


def _build_nc():
    import concourse.bacc as bacc
    import concourse.mybir as mybir
    from concourse.tile import TileContext

    f32, i32, u32 = mybir.dt.float32, mybir.dt.int32, mybir.dt.uint32
    AO, AF, AX = mybir.AluOpType, mybir.ActivationFunctionType, mybir.AxisListType

    nc = bacc.Bacc("TRN2", debug=False, enable_asserts=False)
    lg_d = nc.dram_tensor("logits", [128, T * C], f32, kind="ExternalInput")
    bp_d = nc.dram_tensor("bp", [128, T * W], i32, kind="ExternalOutput")
    fin_d = nc.dram_tensor("fin", [128, W], f32, kind="ExternalOutput")

    with TileContext(nc) as tc:
        with tc.tile_pool(name="pp", bufs=1) as pp, tc.tile_pool(name="sp", bufs=3) as sp:
            D = nc.vector
            A = nc.scalar
            G = nc.gpsimd

            LG = pp.tile([128, T * C], f32)
            LOGP = pp.tile([128, T * C], f32)
            CAND = pp.tile([128, 528], f32)
            BP = pp.tile([128, T * W], i32)
            # f32 state pack: PNB|PB|PTOT|LASTF|ONES|BM|SPB|Z
            PACKF = pp.tile([128, 64], f32)
            # i32 state pack: LAST|NODE|PNODE
            S3 = pp.tile([128, 24], i32)
            IOTA62 = pp.tile([128, 62], i32)
            IOTA62F = pp.tile([128, 62], f32)
            IOTA8 = pp.tile([128, 8], i32)
            EXTV = pp.tile([128, 24], f32)   # [MAXV | NEG | MAXVdup]

            PNB = PACKF[:, 0:8]
            PB = PACKF[:, 8:16]
            PTOT = PACKF[:, 16:24]
            LASTF = PACKF[:, 24:32]
            ONES = PACKF[:, 32:40]
            BM = PACKF[:, 40:48]
            SPB = PACKF[:, 48:56]
            ZS = PACKF[:, 56:64]
            LAST = S3[:, 0:8]
            NODE = S3[:, 8:16]
            PNODE = S3[:, 16:24]

            G.iota(IOTA62[:], pattern=[[1, V]], base=0, channel_multiplier=0)
            G.iota(IOTA8[:], pattern=[[1, 8]], base=0, channel_multiplier=0)
            D.tensor_copy(IOTA62F[:], IOTA62[:])
            D.memset(EXTV[:, 8:16], NEG)
            D.memset(CAND[:], NEG)
            D.memset(ONES, 1.0)

            # log softmax over C, in chunks of TC_ timesteps
            TC_ = 16
            for ch in range(T // TC_):
                a, b = ch * TC_ * C, (ch + 1) * TC_ * C
                lgc = LG[:, a:b]
                nc.sync.dma_start(lgc, lg_d.ap()[:, a:b])
                MXc = pp.tile([128, TC_], f32, name=f"MXc{ch}")
                SMc = pp.tile([128, TC_], f32, name=f"SMc{ch}")
                EXc = pp.tile([128, TC_ * C], f32, name=f"EXc{ch}")
                lg3 = lgc.rearrange("p (t c) -> p t c", c=C)
                lp3 = LOGP[:, a:b].rearrange("p (t c) -> p t c", c=C)
                ex3 = EXc[:].rearrange("p (t c) -> p t c", c=C)
                D.tensor_reduce(MXc[:], lg3, AX.X, AO.max)
                D.tensor_tensor(lp3, lg3, MXc[:].unsqueeze(2).broadcast_to([128, TC_, C]), AO.subtract)
                A.activation(EXc[:], LOGP[:, a:b], AF.Exp)
                D.tensor_reduce(SMc[:], ex3, AX.X, AO.add)
                A.activation(SMc[:], SMc[:], AF.Ln)
                D.tensor_tensor(lp3, lp3, SMc[:].unsqueeze(2).broadcast_to([128, TC_, C]), AO.subtract)

            # state init (ptot kept as state; logaddexp(0,NEG)=0, else NEG)
            D.memset(PB, NEG)
            D.memset(PACKF[:, 8:9], 0.0)
            D.memset(PNB, NEG)
            D.memset(PTOT, NEG)
            D.memset(PACKF[:, 16:17], 0.0)
            D.memset(LAST, -1)
            D.tensor_copy(NODE, IOTA8[:])
            D.memset(PNODE, -1)
            D.memset(LASTF, -1.0)

            def lse(out_ap, a_ap, b_ap, tag):
                """out = max + ln(1 + exp(min - max)) (== logaddexp(a, b))."""
                mxv = sp.tile([128, 8], f32, name=f"mx_{tag}")
                mnv = sp.tile([128, 8], f32, name=f"mn_{tag}")
                D.tensor_tensor(mxv[:], a_ap, b_ap, AO.max)
                D.tensor_tensor(mnv[:], a_ap, b_ap, AO.min)
                D.tensor_tensor(mnv[:], mnv[:], mxv[:], AO.subtract)
                A.activation(mnv[:], mnv[:], AF.Exp)
                A.activation(mnv[:], mnv[:], AF.Ln, bias=1.0)
                D.tensor_tensor(out_ap, mxv[:], mnv[:], AO.add)

            # strided [w, c] views over the 8*64 extend region (pads untouched)
            def wcv(tile_ap):
                return tile_ap.rearrange("p (w c) -> p w c", c=64)[:, :, 0:V]

            ce3 = wcv(CAND[:, 8:520])
            CEFLAT = CAND[:, 8:520]

            for t in range(T):
                lp62 = LOGP[:, t * C: t * C + V]
                lpB = LOGP[:, t * C + V: t * C + C]
                lp62b = lp62.unsqueeze(1).broadcast_to([128, W, V])

                # stay-blank score piece (ptot is maintained state)
                D.tensor_tensor(SPB, PTOT, lpB.broadcast_to([128, 8]), AO.add)

                # last-char one-hot (f32 compare on gpsimd) / lp gather
                ISEQF = sp.tile([128, 512], f32)
                iseqf3 = wcv(ISEQF[:])
                D.tensor_tensor(iseqf3, LAST.unsqueeze(2).broadcast_to([128, W, V]),
                                IOTA62[:].unsqueeze(1).broadcast_to([128, W, V]), AO.is_equal)
                T496 = sp.tile([128, 512], f32)
                D.tensor_tensor(wcv(T496[:]), iseqf3, lp62b, AO.mult)
                LPPICK = sp.tile([128, 8], f32)
                D.tensor_reduce(LPPICK[:], wcv(T496[:]), AX.X, AO.add)
                SPNB = sp.tile([128, 8], f32)
                D.tensor_tensor(SPNB[:], PNB, LPPICK[:], AO.add)

                # extend plane: (is_rep ? pb : ptot) + lp
                PBB = sp.tile([128, 512], f32)
                G.tensor_tensor(wcv(PBB[:]), PB.unsqueeze(2).broadcast_to([128, W, V]),
                                lp62b, AO.add)
                G.tensor_tensor(ce3, PTOT.unsqueeze(2).broadcast_to([128, W, V]),
                                lp62b, AO.add)
                D.copy_predicated(ce3, iseqf3.bitcast(i32), wcv(PBB[:]))

                # parent match (pnode_k == node_i); gather [PB|PNB|PTOT|LASTF|ONES]
                M64 = sp.tile([128, 64], f32)
                m3 = M64[:].rearrange("p (k i) -> p k i", k=W)
                D.tensor_tensor(m3, PNODE.unsqueeze(2).broadcast_to([128, W, W]),
                                NODE.unsqueeze(1).broadcast_to([128, W, W]), AO.is_equal)
                T384 = sp.tile([128, 256], f32)
                t384v = T384[:].rearrange("p (k q i) -> p k q i", k=W, q=4)
                D.tensor_tensor(
                    t384v,
                    m3.unsqueeze(2).broadcast_to([128, W, 4, W]),
                    PACKF[:, 8:40].rearrange("p (q j) -> p q j", q=4).unsqueeze(1).broadcast_to([128, W, 4, W]),
                    AO.mult)
                MG = sp.tile([128, 32], f32)
                mgq = MG[:].rearrange("p (q k) -> p k q", q=4)
                D.tensor_reduce(mgq, t384v, AX.X, AO.add)
                PBP = MG[:, 0:8]
                PTP = MG[:, 8:16]
                LASTPF = MG[:, 16:24]
                MTD = MG[:, 24:32]
                REP = sp.tile([128, 8], i32)
                D.tensor_tensor(REP[:], LASTF, LASTPF, AO.is_equal)
                EB = sp.tile([128, 8], f32)
                D.tensor_copy(EB[:], PTP)
                D.copy_predicated(EB[:], REP[:], PBP)
                EADD = sp.tile([128, 8], f32)
                D.tensor_tensor(EADD[:], EB[:], LPPICK[:], AO.add)
                DLT = sp.tile([128, 8], f32)
                D.tensor_scalar(DLT[:], MTD, 1.0, BIG, AO.subtract, AO.mult)
                EADDM = sp.tile([128, 8], f32)
                D.tensor_tensor(EADDM[:], EADD[:], DLT[:], AO.add)

                # suppression first (doesn't need Z; stale stay lanes can only
                # no-op-replace NEG with NEG or hit measure-zero value ties)
                D.match_replace(CEFLAT, EADDM[:], CEFLAT, NEG)

                # merged stay pnb and stay score (stay slots live at 8+k*64+62)
                lse(BM, SPNB[:], EADDM[:], "e")
                lse(ZS, SPB, BM, "f")
                D.tensor_copy(CAND[:, 8:520].rearrange("p (w c) -> p w c", c=64)[:, :, 62:63].squeeze(2), ZS)

                MAXV = EXTV[:, 0:8]
                D.max(MAXV, CEFLAT)
                BPS = BP[:, t * W:(t + 1) * W]
                D.max_index(BPS.bitcast(u32), MAXV, CEFLAT)
                SEL = BPS.bitcast(i32)

                # decode selection: par = sel>>6 uniformly; ch-lane 62 marks stay
                PAR = sp.tile([128, 8], i32)
                D.tensor_scalar(PAR[:], SEL, 6, None, AO.arith_shift_right)
                EXTI = sp.tile([128, 24], i32)   # [CH | NEWID | NODEP]
                D.tensor_scalar(EXTI[:, 0:8], SEL, 63, None, AO.bitwise_and)
                ISSTAY = sp.tile([128, 8], i32)
                D.tensor_scalar(ISSTAY[:], EXTI[:, 0:8], V, None, AO.is_equal)

                # parent one-hot gathers (f32: [SPB|BM|Z], i32: [LAST|NODE|PNODE])
                PSEL = sp.tile([128, 64], i32)
                ps3 = PSEL[:].rearrange("p (w j) -> p w j", w=W)
                D.tensor_tensor(ps3, PAR[:].unsqueeze(2).broadcast_to([128, W, W]),
                                IOTA8[:].unsqueeze(1).broadcast_to([128, W, W]), AO.is_equal)
                PSELF = sp.tile([128, 64], f32)
                psf3 = PSELF[:].rearrange("p (w j) -> p w j", w=W)
                D.tensor_tensor(psf3, PAR[:].unsqueeze(2).broadcast_to([128, W, W]),
                                IOTA8[:].unsqueeze(1).broadcast_to([128, W, W]), AO.is_equal)
                TF192 = sp.tile([128, 192], f32)
                tf3 = TF192[:].rearrange("p (w q j) -> p w q j", w=W, q=3)
                D.tensor_tensor(
                    tf3,
                    PSELF[:].rearrange("p (w j) -> p w j", w=W).unsqueeze(2).broadcast_to([128, W, 3, W]),
                    PACKF[:, 40:64].rearrange("p (q j) -> p q j", q=3).unsqueeze(1).broadcast_to([128, W, 3, W]),
                    AO.mult)
                FG = sp.tile([128, 24], f32)
                fgq = FG[:].rearrange("p (q w) -> p w q", q=3)
                D.tensor_reduce(fgq, tf3, AX.X, AO.add)
                TI192 = sp.tile([128, 192], i32)
                ti3 = TI192[:].rearrange("p (w q j) -> p w q j", w=W, q=3)
                D.tensor_tensor(
                    ti3,
                    ps3.unsqueeze(2).broadcast_to([128, W, 3, W]),
                    S3[:].rearrange("p (q j) -> p q j", q=3).unsqueeze(1).broadcast_to([128, W, 3, W]),
                    AO.mult)
                IG = sp.tile([128, 24], i32)
                igq = IG[:].rearrange("p (q w) -> p w q", q=3)
                with nc.allow_low_precision(reason="int32 add reduce is exact"):
                    D.tensor_reduce(igq, ti3, AX.X, AO.add)

                # state update (packed selects; ext lanes of stays are dont-care)
                ISS24 = sp.tile([128, 24], i32)
                D.tensor_copy(ISS24[:].rearrange("p (q w) -> p q w", q=3),
                              ISSTAY[:].unsqueeze(1).broadcast_to([128, 3, 8]))
                D.tensor_scalar(EXTI[:, 8:16], SEL, (t + 1) * 1024, None, AO.add)
                D.tensor_copy(EXTI[:, 16:24], IG[:, 8:16])
                D.tensor_copy(EXTV[:, 16:24], MAXV)
                D.tensor_copy(S3[:], EXTI[:])
                D.copy_predicated(S3[:], ISS24[:], IG[:])
                D.tensor_copy(PACKF[:, 0:24], EXTV[:])
                D.copy_predicated(PACKF[:, 0:24], ISS24[:], FG[:])
                D.tensor_copy(LASTF, LAST)

            nc.sync.dma_start(fin_d.ap(), PTOT)
            nc.sync.dma_start(bp_d.ap(), BP[:])

    # The act-table chooser greedily picks the first table containing each
    # function, so alternating Exp/Ln thrashes between two tables (one
    # ~1.3us reload per activation). Restrict Exp and Ln to the combined
    # natural_log_exp_and_others set (real index preserved) so one resident
    # table serves both and the load hoists out of the loop.
    import concourse.bacc as bacc_mod
    _orig_gat = bacc_mod.get_activation_tables

    def _pinned_tables(arch):
        tabs = dict(_orig_gat(arch))
        both = mybir.ActivationFunctionType.Exp, mybir.ActivationFunctionType.Ln
        for name, s in tabs.items():
            if name != "natural_log_exp_and_others":
                tabs[name] = s - set(both)
        return tabs

    bacc_mod.get_activation_tables = _pinned_tables
    try:
        nc.compile()
    finally:
        bacc_mod.get_activation_tables = _orig_gat
    return nc


_NC_CACHE = None


def _get_nc():
    global _NC_CACHE
    if _NC_CACHE is None:
        _NC_CACHE = _build_nc()
    return _NC_CACHE


def _postprocess(bp, fin):
    """bp [rows, T*8] i32 selected slot ids; fin [rows, 8] f32 final ptot."""
    nb = bp.shape[0]
    sel = bp.reshape(nb, T, W)
    par = (sel >> 6).astype(np.int32)
    cl = sel & 63
    ch = np.where(cl == 62, -1, cl).astype(np.int32)
    dec = np.full((nb, T), -1, np.int32)
    lens = np.zeros((nb,), np.int32)
    w = np.zeros((nb,), np.int32)
    ar = np.arange(nb)
    chars = np.empty((nb, T), np.int32)
    for t in range(T - 1, -1, -1):
        chars[:, t] = ch[ar, t, w]
        w = par[ar, t, w]
    for b in range(nb):
        cs = chars[b][chars[b] >= 0]
        dec[b, :len(cs)] = cs
        lens[b] = len(cs)
    prob = np.exp(fin[:, 0].astype(np.float32))
    return dec, lens, prob


def kernel(logits):
    from concourse.bass_utils import run_bass_kernel_spmd

    logits = np.ascontiguousarray(np.asarray(logits, dtype=np.float32))
    assert logits.shape == (B, T, C), logits.shape
    nc = _get_nc()
    in_maps = [
        {"logits": np.ascontiguousarray(logits[c * BC:(c + 1) * BC].reshape(BC, T * C))}
        for c in range(NCORES)
    ]
    res = run_bass_kernel_spmd(nc, in_maps, core_ids=list(range(NCORES)))
    decoded = np.full((B, 1, T), -1, np.int32)
    lengths = np.zeros((B, 1), np.int32)
    probability = np.zeros((B, 1), np.float32)
    for c in range(NCORES):
        out = res.results[c]
        dec, lens, prob = _postprocess(out["bp"], out["fin"])
        decoded[c * BC:(c + 1) * BC, 0] = dec
        lengths[c * BC:(c + 1) * BC, 0] = lens
        probability[c * BC:(c + 1) * BC, 0] = prob
    return decoded, lengths, probability


# revision 18
# speedup vs baseline: 1.0897x; 1.0025x over previous
"""CTC beam search decoder (beam_width=8, top_paths=1) on 8 Trainium2 cores.

Data-parallel: batch 1024 is split 128 rows per core; each core runs an
identical Bass kernel with the 128 batch rows mapped to the 128 SBUF
partitions. Per time step (T=128, sequential) each partition:
  - builds the CTC candidate scores: 8 merged-stay slots (slot w) plus
    8x62 extend slots laid out at 8 + i*64 + c (power-of-2 stride so the
    selection index decodes with shift/and),
  - tracks prefix identity with int32 node ids (pnode_k == node_i gives the
    CTC stay/extend merge; the merged extend slot is suppressed in-place via
    DVE match_replace on its exact fp32 value),
  - selects the top-8 with DVE max/max_index and records the raw selected
    slot indices as backpointers.
The host reconstructs the top-1 label sequence from the backpointers.

Device outputs per core: bp [128, T*8] int32 (selected slot per new beam),
fin [128, 8] fp32 (final log-prob ptot per beam). Host returns
(decoded [B,1,T] i32, lengths [B,1] i32, probability [B,1] f32) matching
the reference tuple.
"""

import sys

for _p in ("/opt/trn_rl_repo", "/root/.axon_site/_ro/trn_rl_repo"):
    if _p not in sys.path:
        sys.path.append(_p)

import numpy as np

B = 1024
T = 128
C = 63
V = C - 1
W = 8
NCORES = 8
BC = B // NCORES
NEG = -1e30
BIG = 1e30


def _build_nc():
    import concourse.bacc as bacc
    import concourse.mybir as mybir
    from concourse.tile import TileContext

    f32, i32, u32 = mybir.dt.float32, mybir.dt.int32, mybir.dt.uint32
    AO, AF, AX = mybir.AluOpType, mybir.ActivationFunctionType, mybir.AxisListType

    nc = bacc.Bacc("TRN2", debug=False, enable_asserts=False)
    lg_d = nc.dram_tensor("logits", [128, T * C], f32, kind="ExternalInput")
    bp_d = nc.dram_tensor("bp", [128, T * W], i32, kind="ExternalOutput")
    fin_d = nc.dram_tensor("fin", [128, W], f32, kind="ExternalOutput")

    with TileContext(nc) as tc:
        with tc.tile_pool(name="pp", bufs=1) as pp, tc.tile_pool(name="sp", bufs=3) as sp:
            D = nc.vector
            A = nc.scalar
            G = nc.gpsimd

            LG = pp.tile([128, T * C], f32)
            LOGP = pp.tile([128, T * C], f32)
            CAND = pp.tile([128, 528], f32)
            BP = pp.tile([128, T * W], i32)
            # f32 state pack (all beam state kept in fp32; ids < 2^24 exact):
            # LASTF|NODEF|PNODEF|BM|SPB|Z|PNB|PB|PTOT|ONES|LASTF2
            PACKF = pp.tile([128, 88], f32)
            IOTA62 = pp.tile([128, 62], i32)
            IOTA62F = pp.tile([128, 62], f32)
            IOTA8 = pp.tile([128, 8], i32)
            EXTV = pp.tile([128, 24], f32)   # [MAXV | NEG | MAXVdup]
            EXTA = pp.tile([128, 24], f32)   # [CHf | NEWIDF | NODEPF]

            LASTF = PACKF[:, 0:8]
            NODEF = PACKF[:, 8:16]
            PNODEF = PACKF[:, 16:24]
            BM = PACKF[:, 24:32]
            SPB = PACKF[:, 32:40]
            ZS = PACKF[:, 40:48]
            PNB = PACKF[:, 48:56]
            PB = PACKF[:, 56:64]
            PTOT = PACKF[:, 64:72]
            ONES = PACKF[:, 72:80]
            LASTF2 = PACKF[:, 80:88]

            G.iota(IOTA62[:], pattern=[[1, V]], base=0, channel_multiplier=0)
            G.iota(IOTA8[:], pattern=[[1, 8]], base=0, channel_multiplier=0)
            D.tensor_copy(IOTA62F[:], IOTA62[:])
            D.memset(EXTV[:, 8:16], NEG)
            D.memset(CAND[:], NEG)
            D.memset(ONES, 1.0)

            # log softmax over C, in chunks of TC_ timesteps
            TC_ = 16
            for ch in range(T // TC_):
                a, b = ch * TC_ * C, (ch + 1) * TC_ * C
                lgc = LG[:, a:b]
                nc.sync.dma_start(lgc, lg_d.ap()[:, a:b])
                MXc = pp.tile([128, TC_], f32, name=f"MXc{ch}")
                SMc = pp.tile([128, TC_], f32, name=f"SMc{ch}")
                EXc = pp.tile([128, TC_ * C], f32, name=f"EXc{ch}")
                lg3 = lgc.rearrange("p (t c) -> p t c", c=C)
                lp3 = LOGP[:, a:b].rearrange("p (t c) -> p t c", c=C)
                ex3 = EXc[:].rearrange("p (t c) -> p t c", c=C)
                D.tensor_reduce(MXc[:], lg3, AX.X, AO.max)
                D.tensor_tensor(lp3, lg3, MXc[:].unsqueeze(2).broadcast_to([128, TC_, C]), AO.subtract)
                A.activation(EXc[:], LOGP[:, a:b], AF.Exp)
                D.tensor_reduce(SMc[:], ex3, AX.X, AO.add)
                A.activation(SMc[:], SMc[:], AF.Ln)
                D.tensor_tensor(lp3, lp3, SMc[:].unsqueeze(2).broadcast_to([128, TC_, C]), AO.subtract)

            # state init (ptot kept as state; logaddexp(0,NEG)=0, else NEG)
            D.memset(PB, NEG)
            D.memset(PACKF[:, 56:57], 0.0)
            D.memset(PNB, NEG)
            D.memset(PTOT, NEG)
            D.memset(PACKF[:, 64:65], 0.0)
            D.tensor_copy(NODEF, IOTA8[:])
            D.memset(PNODEF, -1.0)
            D.memset(LASTF, -1.0)
            D.memset(LASTF2, -1.0)

            def lse(out_ap, a_ap, b_ap, tag):
                """out = max + ln(1 + exp(min - max)) (== logaddexp(a, b))."""
                mxv = sp.tile([128, 8], f32, name=f"mx_{tag}")
                mnv = sp.tile([128, 8], f32, name=f"mn_{tag}")
                D.tensor_tensor(mxv[:], a_ap, b_ap, AO.max)
                D.tensor_tensor(mnv[:], a_ap, b_ap, AO.min)
                D.tensor_tensor(mnv[:], mnv[:], mxv[:], AO.subtract)
                A.activation(mnv[:], mnv[:], AF.Exp)
                A.activation(mnv[:], mnv[:], AF.Ln, bias=1.0)
                D.tensor_tensor(out_ap, mxv[:], mnv[:], AO.add)

            # strided [w, c] views over the 8*64 extend region (pads untouched)
            def wcv(tile_ap):
                return tile_ap.rearrange("p (w c) -> p w c", c=64)[:, :, 0:V]

            ce3 = wcv(CAND[:, 8:520])
            CEFLAT = CAND[:, 8:520]

            for t in range(T):
                lp62 = LOGP[:, t * C: t * C + V]
                lpB = LOGP[:, t * C + V: t * C + C]
                lp62b = lp62.unsqueeze(1).broadcast_to([128, W, V])

                # stay-blank score piece (ptot is maintained state)
                D.tensor_tensor(SPB, PTOT, lpB.broadcast_to([128, 8]), AO.add)

                # last-char one-hot (f32 compare on gpsimd) / lp gather
                ISEQF = sp.tile([128, 512], f32)
                iseqf3 = wcv(ISEQF[:])
                D.tensor_tensor(iseqf3, LASTF.unsqueeze(2).broadcast_to([128, W, V]),
                                IOTA62F[:].unsqueeze(1).broadcast_to([128, W, V]), AO.is_equal)
                T496 = sp.tile([128, 512], f32)
                D.tensor_tensor(wcv(T496[:]), iseqf3, lp62b, AO.mult)
                LPPICK = sp.tile([128, 8], f32)
                D.tensor_reduce(LPPICK[:], wcv(T496[:]), AX.X, AO.add)
                SPNB = sp.tile([128, 8], f32)
                D.tensor_tensor(SPNB[:], PNB, LPPICK[:], AO.add)

                # extend plane: (is_rep ? pb : ptot) + lp
                PBB = sp.tile([128, 512], f32)
                G.tensor_tensor(wcv(PBB[:]), PB.unsqueeze(2).broadcast_to([128, W, V]),
                                lp62b, AO.add)
                G.tensor_tensor(ce3, PTOT.unsqueeze(2).broadcast_to([128, W, V]),
                                lp62b, AO.add)
                D.copy_predicated(ce3, iseqf3.bitcast(i32), wcv(PBB[:]))

                # parent match (pnode_k == node_i); gather [PB|PNB|PTOT|LASTF|ONES]
                M64 = sp.tile([128, 64], f32)
                m3 = M64[:].rearrange("p (k i) -> p k i", k=W)
                D.tensor_tensor(m3, PNODEF.unsqueeze(2).broadcast_to([128, W, W]),
                                NODEF.unsqueeze(1).broadcast_to([128, W, W]), AO.is_equal)
                T384 = sp.tile([128, 256], f32)
                t384v = T384[:].rearrange("p (k q i) -> p k q i", k=W, q=4)
                D.tensor_tensor(
                    t384v,
                    m3.unsqueeze(2).broadcast_to([128, W, 4, W]),
                    PACKF[:, 56:88].rearrange("p (q j) -> p q j", q=4).unsqueeze(1).broadcast_to([128, W, 4, W]),
                    AO.mult)
                MG = sp.tile([128, 32], f32)
                mgq = MG[:].rearrange("p (q k) -> p k q", q=4)
                D.tensor_reduce(mgq, t384v, AX.X, AO.add)
                PBP = MG[:, 0:8]
                PTP = MG[:, 8:16]
                MTD = MG[:, 16:24]
                LASTPF = MG[:, 24:32]
                REP = sp.tile([128, 8], i32)
                D.tensor_tensor(REP[:], LASTF, LASTPF, AO.is_equal)
                EB = sp.tile([128, 8], f32)
                D.tensor_copy(EB[:], PTP)
                D.copy_predicated(EB[:], REP[:], PBP)
                EADD = sp.tile([128, 8], f32)
                D.tensor_tensor(EADD[:], EB[:], LPPICK[:], AO.add)
                DLT = sp.tile([128, 8], f32)
                D.tensor_scalar(DLT[:], MTD, 1.0, BIG, AO.subtract, AO.mult)
                EADDM = sp.tile([128, 8], f32)
                D.tensor_tensor(EADDM[:], EADD[:], DLT[:], AO.add)

                # suppression first (doesn't need Z; stale stay lanes can only
                # no-op-replace NEG with NEG or hit measure-zero value ties)
                D.match_replace(CEFLAT, EADDM[:], CEFLAT, NEG)

                # merged stay pnb and stay score (stay slots live at 8+k*64+62)
                lse(BM, SPNB[:], EADDM[:], "e")
                lse(ZS, SPB, BM, "f")
                D.tensor_copy(CAND[:, 8:520].rearrange("p (w c) -> p w c", c=64)[:, :, 62:63].squeeze(2), ZS)

                MAXV = EXTV[:, 0:8]
                D.max(MAXV, CEFLAT)
                BPS = BP[:, t * W:(t + 1) * W]
                D.max_index(BPS.bitcast(u32), MAXV, CEFLAT)
                SEL = BPS.bitcast(i32)

                # decode selection: par = sel>>6 uniformly; ch-lane 62 marks stay
                PAR = sp.tile([128, 8], i32)
                D.tensor_scalar(PAR[:], SEL, 6, None, AO.arith_shift_right)
                CH = sp.tile([128, 8], i32)
                D.tensor_scalar(CH[:], SEL, 63, None, AO.bitwise_and)
                ISSTAY = sp.tile([128, 8], i32)
                D.tensor_scalar(ISSTAY[:], CH[:], V, None, AO.is_equal)
                D.tensor_copy(EXTA[:, 0:8], CH[:])
                SELF = sp.tile([128, 8], f32)
                D.tensor_copy(SELF[:], SEL)
                D.tensor_scalar(EXTA[:, 8:16], SELF[:], float((t + 1) * 1024), None, AO.add)

                # parent one-hot gather over all six fp32 state quantities
                PSELF = sp.tile([128, 64], f32)
                psf3 = PSELF[:].rearrange("p (w j) -> p w j", w=W)
                D.tensor_tensor(psf3, PAR[:].unsqueeze(2).broadcast_to([128, W, W]),
                                IOTA8[:].unsqueeze(1).broadcast_to([128, W, W]), AO.is_equal)
                TF = sp.tile([128, 384], f32)
                tf3 = TF[:].rearrange("p (w q j) -> p w q j", w=W, q=6)
                D.tensor_tensor(
                    tf3,
                    psf3.unsqueeze(2).broadcast_to([128, W, 6, W]),
                    PACKF[:, 0:48].rearrange("p (q j) -> p q j", q=6).unsqueeze(1).broadcast_to([128, W, 6, W]),
                    AO.mult)
                GG = sp.tile([128, 48], f32)
                ggq = GG[:].rearrange("p (q w) -> p w q", q=6)
                D.tensor_reduce(ggq, tf3, AX.X, AO.add)

                # state update (packed selects; ext lanes of stays are dont-care)
                ISS24 = sp.tile([128, 24], i32)
                D.tensor_copy(ISS24[:].rearrange("p (q w) -> p q w", q=3),
                              ISSTAY[:].unsqueeze(1).broadcast_to([128, 3, 8]))
                D.tensor_copy(EXTA[:, 16:24], GG[:, 8:16])
                D.tensor_copy(EXTV[:, 16:24], MAXV)
                D.tensor_copy(PACKF[:, 0:24], EXTA[:])
                D.copy_predicated(PACKF[:, 0:24], ISS24[:], GG[:, 0:24])
                D.tensor_copy(PACKF[:, 48:72], EXTV[:])
                D.copy_predicated(PACKF[:, 48:72], ISS24[:], GG[:, 24:48])
                D.tensor_copy(LASTF2, LASTF)

            nc.sync.dma_start(fin_d.ap(), PTOT)
            nc.sync.dma_start(bp_d.ap(), BP[:])

    # The act-table chooser greedily picks the first table containing each
    # function, so alternating Exp/Ln thrashes between two tables (one
    # ~1.3us reload per activation). Restrict Exp and Ln to the combined
    # natural_log_exp_and_others set (real index preserved) so one resident
    # table serves both and the load hoists out of the loop.
    import concourse.bacc as bacc_mod
    _orig_gat = bacc_mod.get_activation_tables

    def _pinned_tables(arch):
        tabs = dict(_orig_gat(arch))
        both = mybir.ActivationFunctionType.Exp, mybir.ActivationFunctionType.Ln
        for name, s in tabs.items():
            if name != "natural_log_exp_and_others":
                tabs[name] = s - set(both)
        return tabs

    bacc_mod.get_activation_tables = _pinned_tables
    try:
        nc.compile()
    finally:
        bacc_mod.get_activation_tables = _orig_gat
    return nc


_NC_CACHE = None


def _get_nc():
    global _NC_CACHE
    if _NC_CACHE is None:
        _NC_CACHE = _build_nc()
    return _NC_CACHE


def _postprocess(bp, fin):
    """bp [rows, T*8] i32 selected slot ids; fin [rows, 8] f32 final ptot."""
    nb = bp.shape[0]
    sel = bp.reshape(nb, T, W)
    par = (sel >> 6).astype(np.int32)
    cl = sel & 63
    ch = np.where(cl == 62, -1, cl).astype(np.int32)
    dec = np.full((nb, T), -1, np.int32)
    lens = np.zeros((nb,), np.int32)
    w = np.zeros((nb,), np.int32)
    ar = np.arange(nb)
    chars = np.empty((nb, T), np.int32)
    for t in range(T - 1, -1, -1):
        chars[:, t] = ch[ar, t, w]
        w = par[ar, t, w]
    for b in range(nb):
        cs = chars[b][chars[b] >= 0]
        dec[b, :len(cs)] = cs
        lens[b] = len(cs)
    prob = np.exp(fin[:, 0].astype(np.float32))
    return dec, lens, prob


def kernel(logits):
    from concourse.bass_utils import run_bass_kernel_spmd

    logits = np.ascontiguousarray(np.asarray(logits, dtype=np.float32))
    assert logits.shape == (B, T, C), logits.shape
    nc = _get_nc()
    in_maps = [
        {"logits": np.ascontiguousarray(logits[c * BC:(c + 1) * BC].reshape(BC, T * C))}
        for c in range(NCORES)
    ]
    res = run_bass_kernel_spmd(nc, in_maps, core_ids=list(range(NCORES)))
    decoded = np.full((B, 1, T), -1, np.int32)
    lengths = np.zeros((B, 1), np.int32)
    probability = np.zeros((B, 1), np.float32)
    for c in range(NCORES):
        out = res.results[c]
        dec, lens, prob = _postprocess(out["bp"], out["fin"])
        decoded[c * BC:(c + 1) * BC, 0] = dec
        lengths[c * BC:(c + 1) * BC, 0] = lens
        probability[c * BC:(c + 1) * BC, 0] = prob
    return decoded, lengths, probability
